# revision 8
# baseline (speedup 1.0000x reference)
"""Bass/Tile TP attention kernel for trn2, 8 NeuronCores.

Strategy (sequence-parallel attention):
  - core r owns query rows [512r, 512r+512)
  - RMS-norm own rows -> PE-transpose -> AllGather xq^T (fp16, E-major)
  - column-shard K^T/V projections (512 feature cols each) + RoPE, AllGather
  - Q^T for own rows via full w_q (no collective)
  - scores -> causal mask -> softmax -> P^T -> attn^T = V x P^T
  - out rows = attn^T^T @ w_out, host concatenates row blocks

All matmul layouts keep the contraction dim on partitions (lhsT convention).
"""

import numpy as np

S = 4096
E = 4096
NC = 8
RB = S // NC          # 512 rows per core
P = 128
KO = E // P           # 32 k-tiles
NCH = 8               # 512-wide chunks over S or E
CH = 512
EPS = 1e-6
BASE_THETA = 10000.0

_BUILT = None


def _build_nc():
    import concourse.bass as bass
    import concourse.mybir as mybir
    import concourse.tile as tile
    from concourse import bacc
    from concourse.masks import make_identity

    dt16 = mybir.dt.float16
    dt32 = mybir.dt.float32
    AX = mybir.AxisListType.X
    mult = mybir.AluOpType.mult
    addop = mybir.AluOpType.add
    maxop = mybir.AluOpType.max
    Copy = mybir.ActivationFunctionType.Copy
    Exp = mybir.ActivationFunctionType.Exp

    nc = bacc.Bacc(
        "TRN2", target_bir_lowering=False, debug=False, num_devices=NC)

    # I/O
    x_r = nc.dram_tensor("x_r", [RB, E], dt32, kind="ExternalInput")
    wq = nc.dram_tensor("wq", [E, E], dt32, kind="ExternalInput")
    wk_c = nc.dram_tensor("wk_c", [E, RB], dt32, kind="ExternalInput")
    wv_c = nc.dram_tensor("wv_c", [E, RB], dt32, kind="ExternalInput")
    wo = nc.dram_tensor("wo", [E, E], dt32, kind="ExternalInput")
    scale_d = nc.dram_tensor("scale", [KO, P], dt32, kind="ExternalInput")
    cos_k = nc.dram_tensor("cos_k", [RB, S], dt16, kind="ExternalInput")
    sin_k = nc.dram_tensor("sin_k", [RB, S], dt16, kind="ExternalInput")
    cos_q = nc.dram_tensor("cos_q", [E, RB], dt16, kind="ExternalInput")
    sin_q = nc.dram_tensor("sin_q", [E, RB], dt16, kind="ExternalInput")
    amask = nc.dram_tensor("amask", [RB, S], dt16, kind="ExternalInput")
    swapm_d = nc.dram_tensor("swapm", [P, P], dt16, kind="ExternalInput")
    out_r = nc.dram_tensor("out_r", [RB, E], dt16, kind="ExternalOutput")

    # internal DRAM (collective bounces); outputs Shared for HBM-HBM perf
    xqT_b = nc.dram_tensor("xqT_b", [E, RB], dt16)
    xqT_all = nc.dram_tensor("xqT_all", [NC * E, RB], dt16, addr_space="Shared")
    kT_b = nc.dram_tensor("kT_b", [RB, S], dt16)
    kT_all = nc.dram_tensor("kT_all", [E, S], dt16, addr_space="Shared")
    v_b = nc.dram_tensor("v_b", [S, RB], dt16)
    v_all = nc.dram_tensor("v_all", [NC * S, RB], dt16, addr_space="Shared")
    RG = [list(range(NC))]

    with tile.TileContext(nc) as tc:
        with tc.tile_pool(name="const", bufs=1) as constp, \
             tc.tile_pool(name="psB", bufs=4, space="PSUM") as psB, \
             tc.tile_pool(name="psA", bufs=2, space="PSUM") as psA, \
             tc.tile_pool(name="pstr", bufs=1, space="PSUM") as pstr, \
             tc.tile_pool(name="pssw", bufs=1, space="PSUM") as pssw:

            ident = constp.tile([P, P], dt16, tag="ident")
            make_identity(nc, ident)
            swap_sb = constp.tile([P, P], dt16, tag="swap")
            nc.sync.dma_start(swap_sb, swapm_d[:])
            scale_sb = constp.tile([P, KO], dt32, tag="scale")
            nc.sync.dma_start(scale_sb, scale_d[:].rearrange("ko p -> p ko"))

            with tc.tile_pool(name="qTp", bufs=1) as qTp, \
                 tc.tile_pool(name="attnTp", bufs=1) as attnTp:
                qT = qTp.tile([P, KO, RB], dt16, tag="qT")
                attnT = attnTp.tile([P, KO, RB], dt16, tag="attnT")

                with tc.tile_pool(name="xqTrp", bufs=1) as xqTrp:
                    xqT_r = xqTrp.tile([P, KO, RB], dt16, tag="xqT_r")

                    # ---- stage A: RMS norm of own rows + transpose ----
                    with tc.tile_pool(name="normp", bufs=2) as normp, \
                         tc.tile_pool(name="nstat", bufs=2) as nstat:
                        for t in range(RB // P):
                            x_sb = normp.tile([P, E], dt32, tag="x")
                            nc.sync.dma_start(x_sb, x_r[t * P:(t + 1) * P, :])
                            sq = normp.tile([P, E], dt32, tag="sq")
                            ssum = nstat.tile([P, 1], dt32, tag="ssum")
                            nc.scalar.activation(
                                sq, x_sb, mybir.ActivationFunctionType.Square,
                                accum_out=ssum)
                            s2 = nstat.tile([P, 1], dt32, tag="s2")
                            nc.vector.tensor_scalar(s2, ssum, 1.0 / E, EPS, mult, addop)
                            s3 = nstat.tile([P, 1], dt32, tag="s3")
                            nc.scalar.sqrt(s3, s2)
                            rinv = nstat.tile([P, 1], dt32, tag="rinv")
                            nc.vector.reciprocal(rinv, s3)
                            xq_sb = normp.tile([P, E], dt16, tag="xq")
                            nc.scalar.activation(xq_sb, x_sb, Copy, scale=rinv[:, 0:1])
                            for c in range(KO):
                                pt = pstr.tile([P, P], dt16, tag="tr")
                                nc.tensor.transpose(pt, xq_sb[:, c * P:(c + 1) * P], ident)
                                nc.scalar.copy(xqT_r[:, c, t * P:(t + 1) * P], pt)

                    nc.sync.dma_start(
                        xqT_b[:].rearrange("(ko p) s -> p ko s", p=P), xqT_r[:])
                    nc.gpsimd.collective_compute(
                        "AllGather", mybir.AluOpType.bypass, replica_groups=RG,
                        ins=[xqT_b[:]], outs=[xqT_all[:]])

                    # ---- stage QT: qT = wq^T @ xq^T for own rows (+rope) ----
                    with tc.tile_pool(name="wqs", bufs=3) as wqs, \
                         tc.tile_pool(name="wqs16", bufs=3) as wqs16, \
                         tc.tile_pool(name="ropes", bufs=2) as ropes:
                        for mg in range(8):
                            pss = [psB.tile([P, CH], dt32, tag="mm4", name=f"ps{i}") for i in range(4)]
                            for k in range(KO):
                                wq32 = wqs.tile([P, CH], dt32, tag="wq32")
                                nc.sync.dma_start(
                                    wq32, wq[k * P:(k + 1) * P, mg * CH:(mg + 1) * CH])
                                wq16 = wqs16.tile([P, CH], dt16, tag="wq16")
                                nc.vector.tensor_scalar_mul(wq16, wq32, scale_sb[:, k:k + 1])
                                for m4 in range(4):
                                    nc.tensor.matmul(
                                        pss[m4], lhsT=wq16[:, m4 * P:(m4 + 1) * P],
                                        rhs=xqT_r[:, k, :],
                                        start=(k == 0), stop=(k == KO - 1))
                            for m4 in range(4):
                                m = mg * 4 + m4
                                nc.scalar.copy(qT[:, m, :], pss[m4])
                                sw = pssw.tile([P, CH], dt32, tag="sw")
                                nc.tensor.matmul(sw, lhsT=swap_sb, rhs=qT[:, m, :],
                                                 start=True, stop=True)
                                cq = ropes.tile([P, CH], dt16, tag="cq")
                                nc.sync.dma_start(cq, cos_q[m * P:(m + 1) * P, :])
                                sq_ = ropes.tile([P, CH], dt16, tag="sq")
                                nc.sync.dma_start(sq_, sin_q[m * P:(m + 1) * P, :])
                                t1 = ropes.tile([P, CH], dt16, tag="t1")
                                nc.vector.tensor_tensor(t1, qT[:, m, :], cq, mult)
                                t2 = ropes.tile([P, CH], dt16, tag="t2")
                                nc.vector.tensor_tensor(t2, sw, sq_, mult)
                                nc.vector.tensor_tensor(qT[:, m, :], t1, t2, addop)

                    # ---- stage K: kT_c (+rope); AllGather early ----
                    xqa = xqT_all[:].rearrange("(c ko p) s -> c p ko s", c=NC, p=P)
                    with tc.tile_pool(name="wkp", bufs=1) as wkp, \
                         tc.tile_pool(name="kvs", bufs=2) as kvs, \
                         tc.tile_pool(name="xqs", bufs=1) as xqs, \
                         tc.tile_pool(name="kropes", bufs=2) as kropes:
                        wk16 = wkp.tile([P, KO, RB], dt16, tag="wk16")
                        for k in range(KO):
                            w32a = kvs.tile([P, RB], dt32, tag="wkv32")
                            nc.sync.dma_start(w32a, wk_c[k * P:(k + 1) * P, :])
                            nc.vector.tensor_scalar_mul(
                                wk16[:, k, :], w32a, scale_sb[:, k:k + 1])

                        for n in range(NCH):
                            xq_q = []
                            for qq in range(4):
                                xt = xqs.tile([P, 8, CH], dt16, tag=f"xqch{qq}")
                                nc.sync.dma_start(xt, xqa[n, :, qq * 8:(qq + 1) * 8, :])
                                xq_q.append(xt)
                            # K^T chunk: 4 eo-tiles x this s-chunk
                            pss = [psB.tile([P, CH], dt32, tag="mm4", name=f"ps{i}") for i in range(4)]
                            for k in range(KO):
                                rhs = xq_q[k // 8][:, k % 8, :]
                                for m4 in range(4):
                                    nc.tensor.matmul(
                                        pss[m4], lhsT=wk16[:, k, m4 * P:(m4 + 1) * P],
                                        rhs=rhs, start=(k == 0), stop=(k == KO - 1))
                            for m4 in range(4):
                                kev = kvs.tile([P, CH], dt16, tag="kev")
                                nc.scalar.copy(kev, pss[m4])
                                sw = pssw.tile([P, CH], dt32, tag="sw")
                                nc.tensor.matmul(sw, lhsT=swap_sb, rhs=kev,
                                                 start=True, stop=True)
                                ck = kropes.tile([P, CH], dt16, tag="ck")
                                nc.sync.dma_start(
                                    ck, cos_k[m4 * P:(m4 + 1) * P, n * CH:(n + 1) * CH])
                                sk = kropes.tile([P, CH], dt16, tag="sk")
                                nc.sync.dma_start(
                                    sk, sin_k[m4 * P:(m4 + 1) * P, n * CH:(n + 1) * CH])
                                t1 = kropes.tile([P, CH], dt16, tag="kt1")
                                nc.vector.tensor_tensor(t1, kev, ck, mult)
                                t2 = kropes.tile([P, CH], dt16, tag="kt2")
                                nc.vector.tensor_tensor(t2, sw, sk, mult)
                                kro = kvs.tile([P, CH], dt16, tag="kro")
                                nc.vector.tensor_tensor(kro, t1, t2, addop)
                                nc.sync.dma_start(
                                    kT_b[m4 * P:(m4 + 1) * P, n * CH:(n + 1) * CH], kro)

                        nc.gpsimd.collective_compute(
                            "AllGather", mybir.AluOpType.bypass, replica_groups=RG,
                            ins=[kT_b[:]], outs=[kT_all[:]])

                    # ---- stage V: V_c natural; AllGather ----
                    with tc.tile_pool(name="wvp", bufs=1) as wvp, \
                         tc.tile_pool(name="vevp", bufs=3) as vevp, \
                         tc.tile_pool(name="xqs2", bufs=1) as xqs2:
                        wv16 = wvp.tile([P, KO, RB], dt16, tag="wv16")
                        for k in range(KO):
                            w32b = vevp.tile([P, RB], dt32, tag="wv32")
                            nc.sync.dma_start(w32b, wv_c[k * P:(k + 1) * P, :])
                            nc.vector.tensor_scalar_mul(
                                wv16[:, k, :], w32b, scale_sb[:, k:k + 1])
                        for n in range(NCH):
                            xq_q = []
                            for qq in range(4):
                                xt = xqs2.tile([P, 8, CH], dt16, tag=f"xq2ch{qq}")
                                nc.sync.dma_start(xt, xqa[n, :, qq * 8:(qq + 1) * 8, :])
                                xq_q.append(xt)
                            for ms in range(4):
                                psv = psA.tile([P, CH], dt32, tag="mm2")
                                for k in range(KO):
                                    nc.tensor.matmul(
                                        psv,
                                        lhsT=xq_q[k // 8][:, k % 8, ms * P:(ms + 1) * P],
                                        rhs=wv16[:, k, :],
                                        start=(k == 0), stop=(k == KO - 1))
                                vev = vevp.tile([P, CH], dt16, tag="vev")
                                nc.scalar.copy(vev, psv)
                                st = n * 4 + ms
                                nc.sync.dma_start(v_b[st * P:(st + 1) * P, :], vev)

                        nc.gpsimd.collective_compute(
                            "AllGather", mybir.AluOpType.bypass, replica_groups=RG,
                            ins=[v_b[:]], outs=[v_all[:]])

                # xqT_r freed here
                with tc.tile_pool(name="PTp", bufs=1) as PTp:
                    PT = PTp.tile([P, KO, RB], dt16, tag="PT")

                    # ---- scores + softmax + P^T ----
                    with tc.tile_pool(name="Pp", bufs=1) as Pp, \
                         tc.tile_pool(name="kts", bufs=1) as kts, \
                         tc.tile_pool(name="sstat", bufs=4) as sstat, \
                         tc.tile_pool(name="ams", bufs=4) as ams:
                        Ptile = Pp.tile([P, 4, S], dt16, tag="P")
                        kta = kT_all[:].rearrange("(ko p) s -> p ko s", p=P)
                        for n in range(NCH):
                            kt_q = []
                            for qq in range(4):
                                kt = kts.tile([P, 8, CH], dt16, tag=f"ktch{qq}")
                                nc.sync.dma_start(
                                    kt, kta[:, qq * 8:(qq + 1) * 8, n * CH:(n + 1) * CH])
                                kt_q.append(kt)
                            pss = [psB.tile([P, CH], dt32, tag="mm4", name=f"ps{i}") for i in range(4)]
                            for k in range(KO):
                                rhs = kt_q[k // 8][:, k % 8, :]
                                for m4 in range(4):
                                    nc.tensor.matmul(
                                        pss[m4], lhsT=qT[:, k, m4 * P:(m4 + 1) * P],
                                        rhs=rhs, start=(k == 0), stop=(k == KO - 1))
                            for m4 in range(4):
                                am = ams.tile([P, CH], dt16, tag="am")
                                nc.sync.dma_start(
                                    am, amask[m4 * P:(m4 + 1) * P, n * CH:(n + 1) * CH])
                                nc.vector.scalar_tensor_tensor(
                                    out=Ptile[:, m4, n * CH:(n + 1) * CH],
                                    in0=pss[m4], scalar=1.0 / 64.0, in1=am,
                                    op0=mult, op1=addop)
                        for m in range(4):
                            negmax = sstat.tile([P, 1], dt32, tag="nm")
                            nc.vector.tensor_reduce(
                                negmax, Ptile[:, m, :], axis=AX, op=maxop, negate=True)
                            nc.scalar.activation(
                                Ptile[:, m, :], Ptile[:, m, :], Exp,
                                bias=negmax[:, 0:1], scale=1.0)
                            ssum = sstat.tile([P, 1], dt32, tag="sm")
                            nc.vector.reduce_sum(ssum, Ptile[:, m, :], axis=AX)
                            rinv = sstat.tile([P, 1], dt32, tag="ri")
                            nc.vector.reciprocal(rinv, ssum)
                            nc.scalar.mul(Ptile[:, m, :], Ptile[:, m, :], rinv[:, 0:1])
                            for st in range(KO):
                                pt = pstr.tile([P, P], dt16, tag="tr")
                                nc.tensor.transpose(
                                    pt, Ptile[:, m, st * P:(st + 1) * P], ident)
                                nc.scalar.copy(PT[:, st, m * P:(m + 1) * P], pt)

                    # ---- attn^T = V x P^T ----
                    with tc.tile_pool(name="vs", bufs=2) as vs:
                        va = v_all[:].rearrange("(c ko p) eo -> c p ko eo", c=NC, p=P)
                        for m in range(KO):
                            c, sub = m // 4, m % 4
                            vt = vs.tile([P, KO, P], dt16, tag="vt")
                            nc.sync.dma_start(
                                vt, va[c, :, :, sub * P:(sub + 1) * P])
                            ps = psA.tile([P, CH], dt32, tag="mm2")
                            for k in range(KO):
                                nc.tensor.matmul(
                                    ps, lhsT=vt[:, k, :], rhs=PT[:, k, :],
                                    start=(k == 0), stop=(k == KO - 1))
                            nc.scalar.copy(attnT[:, m, :], ps)

                # ---- out = attn @ wo (rows stay ours) ----
                with tc.tile_pool(name="wos", bufs=3) as wos, \
                     tc.tile_pool(name="wos16", bufs=3) as wos16, \
                     tc.tile_pool(name="oev", bufs=4) as oev:
                    for n in range(NCH):
                        pss = [psB.tile([P, CH], dt32, tag="mm4", name=f"ps{i}") for i in range(4)]
                        for k in range(KO):
                            w32 = wos.tile([P, CH], dt32, tag="wo32")
                            nc.sync.dma_start(
                                w32, wo[k * P:(k + 1) * P, n * CH:(n + 1) * CH])
                            w16 = wos16.tile([P, CH], dt16, tag="wo16")
                            nc.vector.tensor_copy(w16, w32)
                            for mq in range(4):
                                nc.tensor.matmul(
                                    pss[mq], lhsT=attnT[:, k, mq * P:(mq + 1) * P],
                                    rhs=w16, start=(k == 0), stop=(k == KO - 1))
                        for mq in range(4):
                            ot = oev.tile([P, CH], dt16, tag="ot")
                            nc.scalar.copy(ot, pss[mq])
                            nc.sync.dma_start(
                                out_r[mq * P:(mq + 1) * P, n * CH:(n + 1) * CH], ot)

    nc.compile()
    return nc


def _tables():
    pos = np.arange(S, dtype=np.float32)[:, None]
    j = np.arange(E // 2, dtype=np.float32)[None, :]
    theta = pos / np.power(np.float32(BASE_THETA), 2.0 * j / np.float32(E))
    cos_t = np.cos(theta).astype(np.float16)   # (S, E/2)
    sin_t = np.sin(theta).astype(np.float16)
    cosE = np.repeat(cos_t, 2, axis=1).T.copy()            # (E, S)
    sgn = np.where(np.arange(E) % 2 == 0, np.float16(1), np.float16(-1))
    sinE = (np.repeat(sin_t, 2, axis=1) * sgn[None, :]).T.copy()  # (E, S)
    return cosE, sinE


def _prep_in_maps(inputs):
    x = np.ascontiguousarray(np.asarray(inputs["x"], dtype=np.float32))
    w_q = np.ascontiguousarray(np.asarray(inputs["w_q"], dtype=np.float32))
    w_k = np.ascontiguousarray(np.asarray(inputs["w_k"], dtype=np.float32))
    w_v = np.ascontiguousarray(np.asarray(inputs["w_v"], dtype=np.float32))
    w_out = np.ascontiguousarray(np.asarray(inputs["w_out"], dtype=np.float32))
    sf = np.asarray(inputs["scaling_factor"], dtype=np.float32)

    cosE, sinE = _tables()
    swapm = np.zeros((P, P), dtype=np.float16)
    ii = np.arange(0, P, 2)
    swapm[ii, ii + 1] = np.float16(1)
    swapm[ii + 1, ii] = np.float16(1)
    scale_in = np.ascontiguousarray(sf.reshape(KO, P))

    col = np.arange(S)[None, :]
    in_maps = []
    for r in range(NC):
        row = np.arange(r * RB, (r + 1) * RB)[:, None]
        am = np.where(col > row, np.float16(-np.inf), np.float16(0.0)).astype(np.float16)
        in_maps.append({
            "x_r": np.ascontiguousarray(x[r * RB:(r + 1) * RB, :]),
            "wq": w_q,
            "wk_c": np.ascontiguousarray(w_k[:, r * RB:(r + 1) * RB]),
            "wv_c": np.ascontiguousarray(w_v[:, r * RB:(r + 1) * RB]),
            "wo": w_out,
            "scale": scale_in,
            "cos_k": np.ascontiguousarray(cosE[r * RB:(r + 1) * RB, :]),
            "sin_k": np.ascontiguousarray(sinE[r * RB:(r + 1) * RB, :]),
            "cos_q": np.ascontiguousarray(cosE[:, r * RB:(r + 1) * RB]),
            "sin_q": np.ascontiguousarray(sinE[:, r * RB:(r + 1) * RB]),
            "amask": am,
            "swapm": swapm,
        })
    return in_maps


def _run(inputs, trace=False, **kw):
    global _BUILT
    from concourse.bass_utils import run_bass_kernel_spmd
    if _BUILT is None:
        _BUILT = _build_nc()
    in_maps = _prep_in_maps(inputs)
    res = run_bass_kernel_spmd(_BUILT, in_maps, list(range(NC)), trace=trace, **kw)
    out = np.concatenate(
        [np.asarray(res.results[r]["out_r"]) for r in range(NC)], axis=0)
    return out.astype(np.float16), res


def kernel(**inputs):
    out, _ = _run(inputs, trace=False)
    return out


# revision 10
# speedup vs baseline: 1.2382x; 1.2382x over previous
"""Bass/Tile TP attention kernel for trn2, 8 NeuronCores.

Strategy (sequence-parallel attention):
  - core r owns query rows [512r, 512r+512)
  - RMS-norm own rows -> PE-transpose -> AllGather xq^T (fp16, E-major)
  - column-shard K^T/V projections (512 feature cols each) + RoPE, AllGather
  - Q^T for own rows via full w_q (no collective)
  - scores -> causal mask -> softmax -> P^T -> attn^T = V x P^T
  - out rows = attn^T^T @ w_out, host concatenates row blocks

All matmul layouts keep the contraction dim on partitions (lhsT convention).
"""

import numpy as np

S = 4096
E = 4096
NC = 8
RB = S // NC          # 512 rows per core
P = 128
KO = E // P           # 32 k-tiles
NCH = 8               # 512-wide chunks over S or E
CH = 512
EPS = 1e-6
BASE_THETA = 10000.0

_BUILT = None


def _build_nc():
    import concourse.bass as bass
    import concourse.mybir as mybir
    import concourse.tile as tile
    from concourse import bacc
    from concourse.masks import make_identity

    dt16 = mybir.dt.float16
    dt32 = mybir.dt.float32
    AX = mybir.AxisListType.X
    mult = mybir.AluOpType.mult
    addop = mybir.AluOpType.add
    maxop = mybir.AluOpType.max
    Copy = mybir.ActivationFunctionType.Copy
    Exp = mybir.ActivationFunctionType.Exp

    nc = bacc.Bacc(
        "TRN2", target_bir_lowering=False, debug=False, num_devices=NC)

    # I/O
    x_r = nc.dram_tensor("x_r", [RB, E], dt32, kind="ExternalInput")
    wq = nc.dram_tensor("wq", [E, E], dt32, kind="ExternalInput")
    wk_c = nc.dram_tensor("wk_c", [E, RB], dt32, kind="ExternalInput")
    wv_c = nc.dram_tensor("wv_c", [E, RB], dt32, kind="ExternalInput")
    wo = nc.dram_tensor("wo", [E, E], dt32, kind="ExternalInput")
    scale_d = nc.dram_tensor("scale", [KO, P], dt32, kind="ExternalInput")
    cos_k = nc.dram_tensor("cos_k", [RB, S], dt16, kind="ExternalInput")
    sin_k = nc.dram_tensor("sin_k", [RB, S], dt16, kind="ExternalInput")
    cos_q = nc.dram_tensor("cos_q", [E, RB], dt16, kind="ExternalInput")
    sin_q = nc.dram_tensor("sin_q", [E, RB], dt16, kind="ExternalInput")
    amask = nc.dram_tensor("amask", [RB, S], dt16, kind="ExternalInput")
    swapm_d = nc.dram_tensor("swapm", [P, P], dt16, kind="ExternalInput")
    out_r = nc.dram_tensor("out_r", [RB, E], dt16, kind="ExternalOutput")

    # internal DRAM (collective bounces); outputs Shared for HBM-HBM perf
    xqT_b = nc.dram_tensor("xqT_b", [E, RB], dt16)
    xqT_all = nc.dram_tensor("xqT_all", [NC * E, RB], dt16, addr_space="Shared")
    kT_b = nc.dram_tensor("kT_b", [RB, S], dt16)
    kT_all = nc.dram_tensor("kT_all", [E, S], dt16, addr_space="Shared")
    v_b = nc.dram_tensor("v_b", [S, RB], dt16)
    v_all = nc.dram_tensor("v_all", [NC * S, RB], dt16, addr_space="Shared")
    RG = [list(range(NC))]

    with tile.TileContext(nc) as tc:
        with tc.tile_pool(name="const", bufs=1) as constp, \
             tc.tile_pool(name="psB", bufs=4, space="PSUM") as psB, \
             tc.tile_pool(name="psA", bufs=2, space="PSUM") as psA, \
             tc.tile_pool(name="pstr", bufs=1, space="PSUM") as pstr, \
             tc.tile_pool(name="pssw", bufs=1, space="PSUM") as pssw:

            ident = constp.tile([P, P], dt16, tag="ident")
            make_identity(nc, ident)
            swap_sb = constp.tile([P, P], dt16, tag="swap")
            nc.sync.dma_start(swap_sb, swapm_d[:])
            scale_sb = constp.tile([P, KO], dt32, tag="scale")
            nc.sync.dma_start(scale_sb, scale_d[:].rearrange("ko p -> p ko"))

            with tc.tile_pool(name="qTp", bufs=1) as qTp, \
                 tc.tile_pool(name="attnTp", bufs=1) as attnTp:
                qT = qTp.tile([P, KO, RB], dt16, tag="qT")
                attnT = attnTp.tile([P, KO, RB], dt16, tag="attnT")

                with tc.tile_pool(name="xqTrp", bufs=1) as xqTrp:
                    xqT_r = xqTrp.tile([P, KO, RB], dt16, tag="xqT_r")

                    # ---- stage A: RMS norm of own rows + transpose ----
                    with tc.tile_pool(name="normp", bufs=2) as normp, \
                         tc.tile_pool(name="nstat", bufs=2) as nstat:
                        for t in range(RB // P):
                            x_sb = normp.tile([P, E], dt32, tag="x")
                            nc.sync.dma_start(x_sb, x_r[t * P:(t + 1) * P, :])
                            sq = normp.tile([P, E], dt32, tag="sq")
                            ssum = nstat.tile([P, 1], dt32, tag="ssum")
                            nc.scalar.activation(
                                sq, x_sb, mybir.ActivationFunctionType.Square,
                                accum_out=ssum)
                            s2 = nstat.tile([P, 1], dt32, tag="s2")
                            nc.vector.tensor_scalar(s2, ssum, 1.0 / E, EPS, mult, addop)
                            s3 = nstat.tile([P, 1], dt32, tag="s3")
                            nc.scalar.sqrt(s3, s2)
                            rinv = nstat.tile([P, 1], dt32, tag="rinv")
                            nc.vector.reciprocal(rinv, s3)
                            xq_sb = normp.tile([P, E], dt16, tag="xq")
                            nc.scalar.activation(xq_sb, x_sb, Copy, scale=rinv[:, 0:1])
                            for c in range(KO):
                                pt = pstr.tile([P, P], dt16, tag="tr")
                                nc.tensor.transpose(pt, xq_sb[:, c * P:(c + 1) * P], ident)
                                nc.scalar.copy(xqT_r[:, c, t * P:(t + 1) * P], pt)

                    nc.sync.dma_start(
                        xqT_b[:].rearrange("(ko p) s -> p ko s", p=P), xqT_r[:])
                    nc.gpsimd.collective_compute(
                        "AllGather", mybir.AluOpType.bypass, replica_groups=RG,
                        ins=[xqT_b[:]], outs=[xqT_all[:]])

                    # ---- stage QT: qT = wq^T @ xq^T for own rows (+rope) ----
                    with tc.tile_pool(name="wqs", bufs=3) as wqs, \
                         tc.tile_pool(name="wqs16", bufs=3) as wqs16, \
                         tc.tile_pool(name="ropes", bufs=2) as ropes:
                        for mg in range(8):
                            pss = [psB.tile([P, CH], dt32, tag="mm4", name=f"ps{i}") for i in range(4)]
                            for k in range(KO):
                                wq32 = wqs.tile([P, CH], dt32, tag="wq32")
                                nc.sync.dma_start(
                                    wq32, wq[k * P:(k + 1) * P, mg * CH:(mg + 1) * CH])
                                wq16 = wqs16.tile([P, CH], dt16, tag="wq16")
                                nc.vector.tensor_scalar_mul(wq16, wq32, scale_sb[:, k:k + 1])
                                for m4 in range(4):
                                    nc.tensor.matmul(
                                        pss[m4], lhsT=wq16[:, m4 * P:(m4 + 1) * P],
                                        rhs=xqT_r[:, k, :],
                                        start=(k == 0), stop=(k == KO - 1))
                            for m4 in range(4):
                                m = mg * 4 + m4
                                nc.scalar.copy(qT[:, m, :], pss[m4])
                                sw = pssw.tile([P, CH], dt32, tag="sw")
                                nc.tensor.matmul(sw, lhsT=swap_sb, rhs=qT[:, m, :],
                                                 start=True, stop=True)
                                cq = ropes.tile([P, CH], dt16, tag="cq")
                                nc.sync.dma_start(cq, cos_q[m * P:(m + 1) * P, :])
                                sq_ = ropes.tile([P, CH], dt16, tag="sq")
                                nc.sync.dma_start(sq_, sin_q[m * P:(m + 1) * P, :])
                                t1 = ropes.tile([P, CH], dt16, tag="t1")
                                nc.vector.tensor_tensor(t1, qT[:, m, :], cq, mult)
                                t2 = ropes.tile([P, CH], dt16, tag="t2")
                                nc.vector.tensor_tensor(t2, sw, sq_, mult)
                                nc.vector.tensor_tensor(qT[:, m, :], t1, t2, addop)

                    # ---- stage K: kT_c (+rope); AllGather early ----
                    xqa = xqT_all[:].rearrange("(c ko p) s -> c p ko s", c=NC, p=P)
                    with tc.tile_pool(name="wkp", bufs=1) as wkp, \
                         tc.tile_pool(name="kvs", bufs=2) as kvs, \
                         tc.tile_pool(name="xqs", bufs=1) as xqs, \
                         tc.tile_pool(name="kropes", bufs=2) as kropes:
                        wk16 = wkp.tile([P, KO, RB], dt16, tag="wk16")
                        for k in range(KO):
                            w32a = kvs.tile([P, RB], dt32, tag="wkv32")
                            nc.sync.dma_start(w32a, wk_c[k * P:(k + 1) * P, :])
                            nc.vector.tensor_scalar_mul(
                                wk16[:, k, :], w32a, scale_sb[:, k:k + 1])

                        for n in range(NCH):
                            xq_q = []
                            for qq in range(4):
                                xt = xqs.tile([P, 8, CH], dt16, tag=f"xqch{qq}")
                                nc.sync.dma_start(xt, xqa[n, :, qq * 8:(qq + 1) * 8, :])
                                xq_q.append(xt)
                            # K^T chunk: 4 eo-tiles x this s-chunk
                            pss = [psB.tile([P, CH], dt32, tag="mm4", name=f"ps{i}") for i in range(4)]
                            for k in range(KO):
                                rhs = xq_q[k // 8][:, k % 8, :]
                                for m4 in range(4):
                                    nc.tensor.matmul(
                                        pss[m4], lhsT=wk16[:, k, m4 * P:(m4 + 1) * P],
                                        rhs=rhs, start=(k == 0), stop=(k == KO - 1))
                            for m4 in range(4):
                                kev = kvs.tile([P, CH], dt16, tag="kev")
                                nc.scalar.copy(kev, pss[m4])
                                sw = pssw.tile([P, CH], dt32, tag="sw")
                                nc.tensor.matmul(sw, lhsT=swap_sb, rhs=kev,
                                                 start=True, stop=True)
                                ck = kropes.tile([P, CH], dt16, tag="ck")
                                nc.sync.dma_start(
                                    ck, cos_k[m4 * P:(m4 + 1) * P, n * CH:(n + 1) * CH])
                                sk = kropes.tile([P, CH], dt16, tag="sk")
                                nc.sync.dma_start(
                                    sk, sin_k[m4 * P:(m4 + 1) * P, n * CH:(n + 1) * CH])
                                t1 = kropes.tile([P, CH], dt16, tag="kt1")
                                nc.vector.tensor_tensor(t1, kev, ck, mult)
                                t2 = kropes.tile([P, CH], dt16, tag="kt2")
                                nc.vector.tensor_tensor(t2, sw, sk, mult)
                                kro = kvs.tile([P, CH], dt16, tag="kro")
                                nc.vector.tensor_tensor(kro, t1, t2, addop)
                                for jj in range(4):
                                    g = 8 * jj + n  # global 128-col block
                                    nc.sync.dma_start(
                                        kT_b[m4 * P:(m4 + 1) * P, g * P:(g + 1) * P],
                                        kro[:, jj * P:(jj + 1) * P])

                        nc.gpsimd.collective_compute(
                            "AllGather", mybir.AluOpType.bypass, replica_groups=RG,
                            ins=[kT_b[:]], outs=[kT_all[:]])

                    # ---- stage V: V_c natural; AllGather ----
                    with tc.tile_pool(name="wvp", bufs=1) as wvp, \
                         tc.tile_pool(name="vevp", bufs=3) as vevp, \
                         tc.tile_pool(name="xqs2", bufs=1) as xqs2:
                        wv16 = wvp.tile([P, KO, RB], dt16, tag="wv16")
                        for k in range(KO):
                            w32b = vevp.tile([P, RB], dt32, tag="wv32")
                            nc.sync.dma_start(w32b, wv_c[k * P:(k + 1) * P, :])
                            nc.vector.tensor_scalar_mul(
                                wv16[:, k, :], w32b, scale_sb[:, k:k + 1])
                        for n in range(NCH):
                            xq_q = []
                            for qq in range(4):
                                xt = xqs2.tile([P, 8, CH], dt16, tag=f"xq2ch{qq}")
                                nc.sync.dma_start(xt, xqa[n, :, qq * 8:(qq + 1) * 8, :])
                                xq_q.append(xt)
                            for ms in range(4):
                                psv = psA.tile([P, CH], dt32, tag="mm2")
                                for k in range(KO):
                                    nc.tensor.matmul(
                                        psv,
                                        lhsT=xq_q[k // 8][:, k % 8, ms * P:(ms + 1) * P],
                                        rhs=wv16[:, k, :],
                                        start=(k == 0), stop=(k == KO - 1))
                                vev = vevp.tile([P, CH], dt16, tag="vev")
                                nc.scalar.copy(vev, psv)
                                st = 8 * ms + n  # global 128-row block
                                nc.sync.dma_start(v_b[st * P:(st + 1) * P, :], vev)

                        nc.gpsimd.collective_compute(
                            "AllGather", mybir.AluOpType.bypass, replica_groups=RG,
                            ins=[v_b[:]], outs=[v_all[:]])

                # xqT_r freed here
                with tc.tile_pool(name="PTp", bufs=1) as PTp:
                    PT = PTp.tile([P, KO, RB], dt16, tag="PT")

                    # ---- scores + softmax + P^T ----
                    with tc.tile_pool(name="Pp", bufs=1) as Pp, \
                         tc.tile_pool(name="kts", bufs=1) as kts, \
                         tc.tile_pool(name="sstat", bufs=4) as sstat, \
                         tc.tile_pool(name="ams", bufs=4) as ams:
                        Ptile = Pp.tile([P, 4, S], dt16, tag="P")
                        kta = kT_all[:].rearrange("(ko p) s -> p ko s", p=P)
                        for n in range(NCH):
                            # query-tile j only needs key chunks 0..2j+1 (causal)
                            allowed = [m4 for m4 in range(4) if n <= 2 * m4 + 1]
                            kt_q = []
                            for qq in range(4):
                                kt = kts.tile([P, 8, CH], dt16, tag=f"ktch{qq}")
                                nc.sync.dma_start(
                                    kt, kta[:, qq * 8:(qq + 1) * 8, n * CH:(n + 1) * CH])
                                kt_q.append(kt)
                            pss = {m4: psB.tile([P, CH], dt32, tag="mm4", name=f"ps{m4}")
                                   for m4 in allowed}
                            for k in range(KO):
                                rhs = kt_q[k // 8][:, k % 8, :]
                                for m4 in allowed:
                                    nc.tensor.matmul(
                                        pss[m4], lhsT=qT[:, k, m4 * P:(m4 + 1) * P],
                                        rhs=rhs, start=(k == 0), stop=(k == KO - 1))
                            for m4 in allowed:
                                am = ams.tile([P, CH], dt16, tag="am")
                                nc.sync.dma_start(
                                    am, amask[m4 * P:(m4 + 1) * P, n * CH:(n + 1) * CH])
                                nc.vector.scalar_tensor_tensor(
                                    out=Ptile[:, m4, n * CH:(n + 1) * CH],
                                    in0=pss[m4], scalar=1.0 / 64.0, in1=am,
                                    op0=mult, op1=addop)
                        for m in range(4):
                            L = (2 * m + 2) * CH  # causal prefix length
                            negmax = sstat.tile([P, 1], dt32, tag="nm")
                            nc.vector.tensor_reduce(
                                negmax, Ptile[:, m, :L], axis=AX, op=maxop, negate=True)
                            nc.scalar.activation(
                                Ptile[:, m, :L], Ptile[:, m, :L], Exp,
                                bias=negmax[:, 0:1], scale=1.0)
                            ssum = sstat.tile([P, 1], dt32, tag="sm")
                            nc.vector.reduce_sum(ssum, Ptile[:, m, :L], axis=AX)
                            rinv = sstat.tile([P, 1], dt32, tag="ri")
                            nc.vector.reciprocal(rinv, ssum)
                            nc.scalar.mul(Ptile[:, m, :L], Ptile[:, m, :L], rinv[:, 0:1])
                            for st in range(8 * (m + 1)):
                                pt = pstr.tile([P, P], dt16, tag="tr")
                                nc.tensor.transpose(
                                    pt, Ptile[:, m, st * P:(st + 1) * P], ident)
                                nc.scalar.copy(PT[:, st, m * P:(m + 1) * P], pt)

                    # ---- attn^T = V x P^T ----
                    with tc.tile_pool(name="vs", bufs=2) as vs:
                        va = v_all[:].rearrange("(c ko p) eo -> c p ko eo", c=NC, p=P)
                        for m in range(KO):
                            c, sub = m // 4, m % 4
                            vt = vs.tile([P, KO, P], dt16, tag="vt")
                            nc.sync.dma_start(
                                vt, va[c, :, :, sub * P:(sub + 1) * P])
                            ps = psA.tile([P, CH], dt32, tag="mm2")
                            for k in range(KO):
                                j0 = k // 8  # query tiles j >= j0 attend key tile k
                                nc.tensor.matmul(
                                    ps[:, j0 * P:], lhsT=vt[:, k, :],
                                    rhs=PT[:, k, j0 * P:],
                                    start=(k == 0), stop=(k == KO - 1))
                            nc.scalar.copy(attnT[:, m, :], ps)

                # ---- out = attn @ wo (rows stay ours) ----
                with tc.tile_pool(name="wos", bufs=3) as wos, \
                     tc.tile_pool(name="wos16", bufs=3) as wos16, \
                     tc.tile_pool(name="oev", bufs=4) as oev:
                    for n in range(NCH):
                        pss = [psB.tile([P, CH], dt32, tag="mm4", name=f"ps{i}") for i in range(4)]
                        for k in range(KO):
                            w32 = wos.tile([P, CH], dt32, tag="wo32")
                            nc.sync.dma_start(
                                w32, wo[k * P:(k + 1) * P, n * CH:(n + 1) * CH])
                            w16 = wos16.tile([P, CH], dt16, tag="wo16")
                            nc.vector.tensor_copy(w16, w32)
                            for mq in range(4):
                                nc.tensor.matmul(
                                    pss[mq], lhsT=attnT[:, k, mq * P:(mq + 1) * P],
                                    rhs=w16, start=(k == 0), stop=(k == KO - 1))
                        for mq in range(4):
                            ot = oev.tile([P, CH], dt16, tag="ot")
                            nc.scalar.copy(ot, pss[mq])
                            nc.sync.dma_start(
                                out_r[mq * P:(mq + 1) * P, n * CH:(n + 1) * CH], ot)

    nc.compile()
    return nc


def _tables():
    pos = np.arange(S, dtype=np.float32)[:, None]
    j = np.arange(E // 2, dtype=np.float32)[None, :]
    theta = pos / np.power(np.float32(BASE_THETA), 2.0 * j / np.float32(E))
    cos_t = np.cos(theta).astype(np.float16)   # (S, E/2)
    sin_t = np.sin(theta).astype(np.float16)
    cosE = np.repeat(cos_t, 2, axis=1).T.copy()            # (E, S)
    sgn = np.where(np.arange(E) % 2 == 0, np.float16(1), np.float16(-1))
    sinE = (np.repeat(sin_t, 2, axis=1) * sgn[None, :]).T.copy()  # (E, S)
    return cosE, sinE


def _own_rows(r):
    # core r owns 128-row blocks {8j + r : j=0..3}
    return np.concatenate(
        [np.arange(128 * (8 * j + r), 128 * (8 * j + r) + 128) for j in range(4)])


def _prep_in_maps(inputs):
    x = np.ascontiguousarray(np.asarray(inputs["x"], dtype=np.float32))
    w_q = np.ascontiguousarray(np.asarray(inputs["w_q"], dtype=np.float32))
    w_k = np.ascontiguousarray(np.asarray(inputs["w_k"], dtype=np.float32))
    w_v = np.ascontiguousarray(np.asarray(inputs["w_v"], dtype=np.float32))
    w_out = np.ascontiguousarray(np.asarray(inputs["w_out"], dtype=np.float32))
    sf = np.asarray(inputs["scaling_factor"], dtype=np.float32)

    cosE, sinE = _tables()
    swapm = np.zeros((P, P), dtype=np.float16)
    ii = np.arange(0, P, 2)
    swapm[ii, ii + 1] = np.float16(1)
    swapm[ii + 1, ii] = np.float16(1)
    scale_in = np.ascontiguousarray(sf.reshape(KO, P))

    col = np.arange(S)[None, :]
    # xqT AllGather chunk n holds core n's scattered rows; K-rope tables must
    # follow that column order
    perm = np.concatenate([_own_rows(n) for n in range(NC)])
    in_maps = []
    for r in range(NC):
        rows = _own_rows(r)
        row = rows[:, None]
        am = np.where(col > row, np.float16(-np.inf), np.float16(0.0)).astype(np.float16)
        in_maps.append({
            "x_r": np.ascontiguousarray(x[rows, :]),
            "wq": w_q,
            "wk_c": np.ascontiguousarray(w_k[:, r * RB:(r + 1) * RB]),
            "wv_c": np.ascontiguousarray(w_v[:, r * RB:(r + 1) * RB]),
            "wo": w_out,
            "scale": scale_in,
            "cos_k": np.ascontiguousarray(cosE[r * RB:(r + 1) * RB][:, perm]),
            "sin_k": np.ascontiguousarray(sinE[r * RB:(r + 1) * RB][:, perm]),
            "cos_q": np.ascontiguousarray(cosE[:, rows]),
            "sin_q": np.ascontiguousarray(sinE[:, rows]),
            "amask": am,
            "swapm": swapm,
        })
    return in_maps


def _run(inputs, trace=False, **kw):
    global _BUILT
    from concourse.bass_utils import run_bass_kernel_spmd
    if _BUILT is None:
        _BUILT = _build_nc()
    in_maps = _prep_in_maps(inputs)
    res = run_bass_kernel_spmd(_BUILT, in_maps, list(range(NC)), trace=trace, **kw)
    out = np.empty((S, E), dtype=np.float16)
    for r in range(NC):
        out[_own_rows(r)] = np.asarray(res.results[r]["out_r"]).astype(np.float16)
    return out, res


def kernel(**inputs):
    out, _ = _run(inputs, trace=False)
    return out


# revision 11
# speedup vs baseline: 2.2953x; 1.8538x over previous
"""Bass/Tile TP attention kernel for trn2, 8 NeuronCores.

Strategy (sequence-parallel attention):
  - core r owns query rows [512r, 512r+512)
  - RMS-norm own rows -> PE-transpose -> AllGather xq^T (fp16, E-major)
  - column-shard K^T/V projections (512 feature cols each) + RoPE, AllGather
  - Q^T for own rows via full w_q (no collective)
  - scores -> causal mask -> softmax -> P^T -> attn^T = V x P^T
  - out rows = attn^T^T @ w_out, host concatenates row blocks

All matmul layouts keep the contraction dim on partitions (lhsT convention).
"""

import numpy as np

S = 4096
E = 4096
NC = 8
RB = S // NC          # 512 rows per core
P = 128
KO = E // P           # 32 k-tiles
NCH = 8               # 512-wide chunks over S or E
CH = 512
EPS = 1e-6
BASE_THETA = 10000.0

_BUILT = None


def _build_nc():
    import concourse.bass as bass
    import concourse.mybir as mybir
    import concourse.tile as tile
    from concourse import bacc
    from concourse.masks import make_identity

    dt16 = mybir.dt.float16
    dt32 = mybir.dt.float32
    AX = mybir.AxisListType.X
    mult = mybir.AluOpType.mult
    addop = mybir.AluOpType.add
    maxop = mybir.AluOpType.max
    Copy = mybir.ActivationFunctionType.Copy
    Exp = mybir.ActivationFunctionType.Exp

    nc = bacc.Bacc(
        "TRN2", target_bir_lowering=False, debug=False, num_devices=NC)

    # I/O
    x_r = nc.dram_tensor("x_r", [RB, E], dt32, kind="ExternalInput")
    wq = nc.dram_tensor("wq", [E, E], dt32, kind="ExternalInput")
    wk_c = nc.dram_tensor("wk_c", [E, RB], dt32, kind="ExternalInput")
    wv_c = nc.dram_tensor("wv_c", [E, RB], dt32, kind="ExternalInput")
    wo = nc.dram_tensor("wo", [E, E], dt32, kind="ExternalInput")
    scale_d = nc.dram_tensor("scale", [KO, P], dt32, kind="ExternalInput")
    cos_k = nc.dram_tensor("cos_k", [RB, S], dt16, kind="ExternalInput")
    sin_k = nc.dram_tensor("sin_k", [RB, S], dt16, kind="ExternalInput")
    cos_q = nc.dram_tensor("cos_q", [E, RB], dt16, kind="ExternalInput")
    sin_q = nc.dram_tensor("sin_q", [E, RB], dt16, kind="ExternalInput")
    amask = nc.dram_tensor("amask", [RB, S], dt16, kind="ExternalInput")
    swapm_d = nc.dram_tensor("swapm", [P, P], dt16, kind="ExternalInput")
    out_r = nc.dram_tensor("out_r", [RB, E], dt16, kind="ExternalOutput")

    # internal DRAM (collective bounces); outputs Shared for HBM-HBM perf
    xqT_b = nc.dram_tensor("xqT_b", [E, RB], dt16)
    xqT_all = nc.dram_tensor("xqT_all", [NC * E, RB], dt16, addr_space="Shared")
    kT_b = nc.dram_tensor("kT_b", [RB, S], dt16)
    kT_all = nc.dram_tensor("kT_all", [E, S], dt16, addr_space="Shared")
    v_b = nc.dram_tensor("v_b", [S, RB], dt16)
    v_all = nc.dram_tensor("v_all", [NC * S, RB], dt16, addr_space="Shared")
    RG = [list(range(NC))]

    with tile.TileContext(nc) as tc:
        with tc.tile_pool(name="const", bufs=1) as constp, \
             tc.tile_pool(name="psB", bufs=4, space="PSUM") as psB, \
             tc.tile_pool(name="psA", bufs=1, space="PSUM") as psA, \
             tc.tile_pool(name="pstr", bufs=2, space="PSUM") as pstr, \
             tc.tile_pool(name="pssw", bufs=1, space="PSUM") as pssw:

            ident = constp.tile([P, P], dt16, tag="ident")
            make_identity(nc, ident)
            swap_sb = constp.tile([P, P], dt16, tag="swap")
            nc.sync.dma_start(swap_sb, swapm_d[:])
            scale_sb = constp.tile([P, KO], dt32, tag="scale")
            nc.sync.dma_start(scale_sb, scale_d[:].rearrange("ko p -> p ko"))

            with tc.tile_pool(name="qTp", bufs=1) as qTp, \
                 tc.tile_pool(name="attnTp", bufs=1) as attnTp:
                qT = qTp.tile([P, KO, RB], dt16, tag="qT")
                attnT = attnTp.tile([P, KO, RB], dt16, tag="attnT")

                with tc.tile_pool(name="xqTrp", bufs=1) as xqTrp:
                    xqT_r = xqTrp.tile([P, KO, RB], dt16, tag="xqT_r")

                    # ---- stage A: RMS norm of own rows + transpose ----
                    with tc.tile_pool(name="normp", bufs=2) as normp, \
                         tc.tile_pool(name="nstat", bufs=2) as nstat:
                        for t in range(RB // P):
                            x_sb = normp.tile([P, E], dt32, tag="x")
                            nc.sync.dma_start(x_sb, x_r[t * P:(t + 1) * P, :])
                            sq = normp.tile([P, E], dt32, tag="sq")
                            ssum = nstat.tile([P, 1], dt32, tag="ssum")
                            nc.scalar.activation(
                                sq, x_sb, mybir.ActivationFunctionType.Square,
                                accum_out=ssum)
                            s2 = nstat.tile([P, 1], dt32, tag="s2")
                            nc.vector.tensor_scalar(s2, ssum, 1.0 / E, EPS, mult, addop)
                            s3 = nstat.tile([P, 1], dt32, tag="s3")
                            nc.scalar.sqrt(s3, s2)
                            rinv = nstat.tile([P, 1], dt32, tag="rinv")
                            nc.vector.reciprocal(rinv, s3)
                            xq_sb = normp.tile([P, E], dt16, tag="xq")
                            nc.scalar.activation(xq_sb, x_sb, Copy, scale=rinv[:, 0:1])
                            for c in range(KO):
                                pt = pstr.tile([P, P], dt16, tag="tr")
                                nc.tensor.transpose(pt, xq_sb[:, c * P:(c + 1) * P], ident)
                                nc.scalar.copy(xqT_r[:, c, t * P:(t + 1) * P], pt)

                    nc.sync.dma_start(
                        xqT_b[:].rearrange("(ko p) s -> p ko s", p=P), xqT_r[:])
                    nc.gpsimd.collective_compute(
                        "AllGather", mybir.AluOpType.bypass, replica_groups=RG,
                        ins=[xqT_b[:]], outs=[xqT_all[:]])

                    # ---- stage QT: qT = wq^T @ xq^T for own rows (+rope) ----
                    with tc.tile_pool(name="wqs", bufs=6) as wqs, \
                         tc.tile_pool(name="wqs16", bufs=6) as wqs16, \
                         tc.tile_pool(name="ropes", bufs=2) as ropes:
                        for mg in range(8):
                            pss = [psB.tile([P, CH], dt32, tag="mm4", name=f"ps{i}") for i in range(4)]
                            for k in range(KO):
                                wq32 = wqs.tile([P, CH], dt32, tag="wq32")
                                nc.sync.dma_start(
                                    wq32, wq[k * P:(k + 1) * P, mg * CH:(mg + 1) * CH])
                                wq16 = wqs16.tile([P, CH], dt16, tag="wq16")
                                nc.vector.tensor_scalar_mul(wq16, wq32, scale_sb[:, k:k + 1])
                                for m4 in range(4):
                                    nc.tensor.matmul(
                                        pss[m4], lhsT=wq16[:, m4 * P:(m4 + 1) * P],
                                        rhs=xqT_r[:, k, :],
                                        start=(k == 0), stop=(k == KO - 1))
                            for m4 in range(4):
                                m = mg * 4 + m4
                                nc.scalar.copy(qT[:, m, :], pss[m4])
                                sw = pssw.tile([P, CH], dt32, tag="sw")
                                nc.tensor.matmul(sw, lhsT=swap_sb, rhs=qT[:, m, :],
                                                 start=True, stop=True)
                                cq = ropes.tile([P, CH], dt16, tag="cq")
                                nc.sync.dma_start(cq, cos_q[m * P:(m + 1) * P, :])
                                sq_ = ropes.tile([P, CH], dt16, tag="sq")
                                nc.sync.dma_start(sq_, sin_q[m * P:(m + 1) * P, :])
                                t1 = ropes.tile([P, CH], dt16, tag="t1")
                                nc.vector.tensor_tensor(t1, qT[:, m, :], cq, mult)
                                t2 = ropes.tile([P, CH], dt16, tag="t2")
                                nc.vector.tensor_tensor(t2, sw, sq_, mult)
                                nc.vector.tensor_tensor(qT[:, m, :], t1, t2, addop)

                    # ---- stage K: kT_c (+rope); AllGather early ----
                    xqa = xqT_all[:].rearrange("(c ko p) s -> c p ko s", c=NC, p=P)
                    with tc.tile_pool(name="wkp", bufs=1) as wkp, \
                         tc.tile_pool(name="kvs", bufs=2) as kvs, \
                         tc.tile_pool(name="xqs", bufs=1) as xqs, \
                         tc.tile_pool(name="kropes", bufs=2) as kropes:
                        wk16 = wkp.tile([P, KO, RB], dt16, tag="wk16")
                        for k in range(KO):
                            w32a = kvs.tile([P, RB], dt32, tag="wkv32")
                            nc.sync.dma_start(w32a, wk_c[k * P:(k + 1) * P, :])
                            nc.vector.tensor_scalar_mul(
                                wk16[:, k, :], w32a, scale_sb[:, k:k + 1])

                        for n in range(NCH):
                            xq_q = []
                            for qq in range(4):
                                xt = xqs.tile([P, 8, CH], dt16, tag=f"xqch{qq}")
                                nc.sync.dma_start(xt, xqa[n, :, qq * 8:(qq + 1) * 8, :])
                                xq_q.append(xt)
                            # K^T chunk: 4 eo-tiles x this s-chunk
                            pss = [psB.tile([P, CH], dt32, tag="mm4", name=f"ps{i}") for i in range(4)]
                            for k in range(KO):
                                rhs = xq_q[k // 8][:, k % 8, :]
                                for m4 in range(4):
                                    nc.tensor.matmul(
                                        pss[m4], lhsT=wk16[:, k, m4 * P:(m4 + 1) * P],
                                        rhs=rhs, start=(k == 0), stop=(k == KO - 1))
                            for m4 in range(4):
                                kev = kvs.tile([P, CH], dt16, tag="kev")
                                nc.scalar.copy(kev, pss[m4])
                                sw = pssw.tile([P, CH], dt32, tag="sw")
                                nc.tensor.matmul(sw, lhsT=swap_sb, rhs=kev,
                                                 start=True, stop=True)
                                ck = kropes.tile([P, CH], dt16, tag="ck")
                                nc.sync.dma_start(
                                    ck, cos_k[m4 * P:(m4 + 1) * P, n * CH:(n + 1) * CH])
                                sk = kropes.tile([P, CH], dt16, tag="sk")
                                nc.sync.dma_start(
                                    sk, sin_k[m4 * P:(m4 + 1) * P, n * CH:(n + 1) * CH])
                                t1 = kropes.tile([P, CH], dt16, tag="kt1")
                                nc.vector.tensor_tensor(t1, kev, ck, mult)
                                t2 = kropes.tile([P, CH], dt16, tag="kt2")
                                nc.vector.tensor_tensor(t2, sw, sk, mult)
                                kro = kvs.tile([P, CH], dt16, tag="kro")
                                nc.vector.tensor_tensor(kro, t1, t2, addop)
                                for jj in range(4):
                                    g = 8 * jj + n  # global 128-col block
                                    nc.sync.dma_start(
                                        kT_b[m4 * P:(m4 + 1) * P, g * P:(g + 1) * P],
                                        kro[:, jj * P:(jj + 1) * P])

                        nc.gpsimd.collective_compute(
                            "AllGather", mybir.AluOpType.bypass, replica_groups=RG,
                            ins=[kT_b[:]], outs=[kT_all[:]])

                    # ---- stage V: V_c natural; AllGather ----
                    with tc.tile_pool(name="wvp", bufs=1) as wvp, \
                         tc.tile_pool(name="vevp", bufs=3) as vevp, \
                         tc.tile_pool(name="xqs2", bufs=1) as xqs2:
                        wv16 = wvp.tile([P, KO, RB], dt16, tag="wv16")
                        for k in range(KO):
                            w32b = vevp.tile([P, RB], dt32, tag="wv32")
                            nc.sync.dma_start(w32b, wv_c[k * P:(k + 1) * P, :])
                            nc.vector.tensor_scalar_mul(
                                wv16[:, k, :], w32b, scale_sb[:, k:k + 1])
                        for n in range(NCH):
                            xq_q = []
                            for qq in range(4):
                                xt = xqs2.tile([P, 8, CH], dt16, tag=f"xq2ch{qq}")
                                nc.sync.dma_start(xt, xqa[n, :, qq * 8:(qq + 1) * 8, :])
                                xq_q.append(xt)
                            for ms in range(4):
                                psv = psA.tile([P, CH], dt32, tag="mm2")
                                for k in range(KO):
                                    nc.tensor.matmul(
                                        psv,
                                        lhsT=xq_q[k // 8][:, k % 8, ms * P:(ms + 1) * P],
                                        rhs=wv16[:, k, :],
                                        start=(k == 0), stop=(k == KO - 1))
                                vev = vevp.tile([P, CH], dt16, tag="vev")
                                nc.scalar.copy(vev, psv)
                                st = 8 * ms + n  # global 128-row block
                                nc.sync.dma_start(v_b[st * P:(st + 1) * P, :], vev)

                        nc.gpsimd.collective_compute(
                            "AllGather", mybir.AluOpType.bypass, replica_groups=RG,
                            ins=[v_b[:]], outs=[v_all[:]])

                # xqT_r freed here
                with tc.tile_pool(name="PTp", bufs=1) as PTp:
                    PT = PTp.tile([P, KO, RB], dt16, tag="PT")

                    # ---- scores + softmax + P^T ----
                    with tc.tile_pool(name="Pp", bufs=1) as Pp, \
                         tc.tile_pool(name="kts", bufs=1) as kts, \
                         tc.tile_pool(name="sstat", bufs=4) as sstat, \
                         tc.tile_pool(name="ams", bufs=4) as ams:
                        Ptile = Pp.tile([P, 4, S], dt16, tag="P")
                        kta = kT_all[:].rearrange("(ko p) s -> p ko s", p=P)
                        for n in range(NCH):
                            # query-tile j only needs key chunks 0..2j+1 (causal)
                            allowed = [m4 for m4 in range(4) if n <= 2 * m4 + 1]
                            kt_q = []
                            for qq in range(4):
                                kt = kts.tile([P, 8, CH], dt16, tag=f"ktch{qq}")
                                nc.sync.dma_start(
                                    kt, kta[:, qq * 8:(qq + 1) * 8, n * CH:(n + 1) * CH])
                                kt_q.append(kt)
                            pss = {m4: psB.tile([P, CH], dt32, tag="mm4", name=f"ps{m4}")
                                   for m4 in allowed}
                            for k in range(KO):
                                rhs = kt_q[k // 8][:, k % 8, :]
                                for m4 in allowed:
                                    nc.tensor.matmul(
                                        pss[m4], lhsT=qT[:, k, m4 * P:(m4 + 1) * P],
                                        rhs=rhs, start=(k == 0), stop=(k == KO - 1))
                            for m4 in allowed:
                                am = ams.tile([P, CH], dt16, tag="am")
                                nc.sync.dma_start(
                                    am, amask[m4 * P:(m4 + 1) * P, n * CH:(n + 1) * CH])
                                nc.vector.scalar_tensor_tensor(
                                    out=Ptile[:, m4, n * CH:(n + 1) * CH],
                                    in0=pss[m4], scalar=1.0 / 64.0, in1=am,
                                    op0=mult, op1=addop)
                        for m in range(4):
                            L = (2 * m + 2) * CH  # causal prefix length
                            negmax = sstat.tile([P, 1], dt32, tag="nm")
                            nc.vector.tensor_reduce(
                                negmax, Ptile[:, m, :L], axis=AX, op=maxop, negate=True)
                            nc.scalar.activation(
                                Ptile[:, m, :L], Ptile[:, m, :L], Exp,
                                bias=negmax[:, 0:1], scale=1.0)
                            ssum = sstat.tile([P, 1], dt32, tag="sm")
                            nc.vector.reduce_sum(ssum, Ptile[:, m, :L], axis=AX)
                            rinv = sstat.tile([P, 1], dt32, tag="ri")
                            nc.vector.reciprocal(rinv, ssum)
                            nc.scalar.mul(Ptile[:, m, :L], Ptile[:, m, :L], rinv[:, 0:1])
                            for st in range(8 * (m + 1)):
                                pt = pstr.tile([P, P], dt16, tag="tr")
                                nc.tensor.transpose(
                                    pt, Ptile[:, m, st * P:(st + 1) * P], ident)
                                nc.scalar.copy(PT[:, st, m * P:(m + 1) * P], pt)

                    # ---- attn^T = V x P^T ----
                    with tc.tile_pool(name="vs", bufs=2) as vs:
                        va = v_all[:].rearrange("(c ko p) eo -> c p ko eo", c=NC, p=P)
                        for m in range(KO):
                            c, sub = m // 4, m % 4
                            vt = vs.tile([P, KO, P], dt16, tag="vt")
                            nc.sync.dma_start(
                                vt, va[c, :, :, sub * P:(sub + 1) * P])
                            ps = psA.tile([P, CH], dt32, tag="mm2")
                            for k in range(KO):
                                j0 = k // 8  # query tiles j >= j0 attend key tile k
                                nc.tensor.matmul(
                                    ps[:, j0 * P:], lhsT=vt[:, k, :],
                                    rhs=PT[:, k, j0 * P:],
                                    start=(k == 0), stop=(k == KO - 1))
                            nc.scalar.copy(attnT[:, m, :], ps)

                # ---- out = attn @ wo (rows stay ours) ----
                with tc.tile_pool(name="wos", bufs=6) as wos, \
                     tc.tile_pool(name="wos16", bufs=6) as wos16, \
                     tc.tile_pool(name="oev", bufs=4) as oev:
                    for n in range(NCH):
                        pss = [psB.tile([P, CH], dt32, tag="mm4", name=f"ps{i}") for i in range(4)]
                        for k in range(KO):
                            w32 = wos.tile([P, CH], dt32, tag="wo32")
                            nc.sync.dma_start(
                                w32, wo[k * P:(k + 1) * P, n * CH:(n + 1) * CH])
                            w16 = wos16.tile([P, CH], dt16, tag="wo16")
                            nc.vector.tensor_copy(w16, w32)
                            for mq in range(4):
                                nc.tensor.matmul(
                                    pss[mq], lhsT=attnT[:, k, mq * P:(mq + 1) * P],
                                    rhs=w16, start=(k == 0), stop=(k == KO - 1))
                        for mq in range(4):
                            ot = oev.tile([P, CH], dt16, tag="ot")
                            nc.scalar.copy(ot, pss[mq])
                            nc.sync.dma_start(
                                out_r[mq * P:(mq + 1) * P, n * CH:(n + 1) * CH], ot)

    nc.compile()
    return nc


def _tables():
    pos = np.arange(S, dtype=np.float32)[:, None]
    j = np.arange(E // 2, dtype=np.float32)[None, :]
    theta = pos / np.power(np.float32(BASE_THETA), 2.0 * j / np.float32(E))
    cos_t = np.cos(theta).astype(np.float16)   # (S, E/2)
    sin_t = np.sin(theta).astype(np.float16)
    cosE = np.repeat(cos_t, 2, axis=1).T.copy()            # (E, S)
    sgn = np.where(np.arange(E) % 2 == 0, np.float16(1), np.float16(-1))
    sinE = (np.repeat(sin_t, 2, axis=1) * sgn[None, :]).T.copy()  # (E, S)
    return cosE, sinE


def _own_rows(r):
    # core r owns 128-row blocks {8j + r : j=0..3}
    return np.concatenate(
        [np.arange(128 * (8 * j + r), 128 * (8 * j + r) + 128) for j in range(4)])


def _prep_in_maps(inputs):
    x = np.ascontiguousarray(np.asarray(inputs["x"], dtype=np.float32))
    w_q = np.ascontiguousarray(np.asarray(inputs["w_q"], dtype=np.float32))
    w_k = np.ascontiguousarray(np.asarray(inputs["w_k"], dtype=np.float32))
    w_v = np.ascontiguousarray(np.asarray(inputs["w_v"], dtype=np.float32))
    w_out = np.ascontiguousarray(np.asarray(inputs["w_out"], dtype=np.float32))
    sf = np.asarray(inputs["scaling_factor"], dtype=np.float32)

    cosE, sinE = _tables()
    swapm = np.zeros((P, P), dtype=np.float16)
    ii = np.arange(0, P, 2)
    swapm[ii, ii + 1] = np.float16(1)
    swapm[ii + 1, ii] = np.float16(1)
    scale_in = np.ascontiguousarray(sf.reshape(KO, P))

    col = np.arange(S)[None, :]
    # xqT AllGather chunk n holds core n's scattered rows; K-rope tables must
    # follow that column order
    perm = np.concatenate([_own_rows(n) for n in range(NC)])
    in_maps = []
    for r in range(NC):
        rows = _own_rows(r)
        row = rows[:, None]
        am = np.where(col > row, np.float16(-np.inf), np.float16(0.0)).astype(np.float16)
        in_maps.append({
            "x_r": np.ascontiguousarray(x[rows, :]),
            "wq": w_q,
            "wk_c": np.ascontiguousarray(w_k[:, r * RB:(r + 1) * RB]),
            "wv_c": np.ascontiguousarray(w_v[:, r * RB:(r + 1) * RB]),
            "wo": w_out,
            "scale": scale_in,
            "cos_k": np.ascontiguousarray(cosE[r * RB:(r + 1) * RB][:, perm]),
            "sin_k": np.ascontiguousarray(sinE[r * RB:(r + 1) * RB][:, perm]),
            "cos_q": np.ascontiguousarray(cosE[:, rows]),
            "sin_q": np.ascontiguousarray(sinE[:, rows]),
            "amask": am,
            "swapm": swapm,
        })
    return in_maps


def _run(inputs, trace=False, **kw):
    global _BUILT
    from concourse.bass_utils import run_bass_kernel_spmd
    if _BUILT is None:
        _BUILT = _build_nc()
    in_maps = _prep_in_maps(inputs)
    res = run_bass_kernel_spmd(_BUILT, in_maps, list(range(NC)), trace=trace, **kw)
    out = np.empty((S, E), dtype=np.float16)
    for r in range(NC):
        out[_own_rows(r)] = np.asarray(res.results[r]["out_r"]).astype(np.float16)
    return out, res


def kernel(**inputs):
    out, _ = _run(inputs, trace=False)
    return out
